# revision 5
# baseline (speedup 1.0000x reference)
"""Dense multi-head attention (DotProductAttention) for Trainium2, 8-core SPMD.

Full inputs: query/key/value [b=2, s=2048, nh=32, hn=64] fp32.
Sharding: b*nh = 64 head-units split across 8 cores (8 units/core),
each core computes full attention for its units, no cross-core comms.

v4: software-pipelined rewrite. HW findings that drive it (microbench
via loop-count slope on this container's TRN2):

  1. ScalarE ACTIVATE cost depends strongly on OUTPUT dtype: f32r
     2542ns / bf16 2435ns / f32 927ns / fp16 567ns per [128,1024]
     PSUM->SBUF exp. The staged baseline's pT was f32r-typed, so its
     exp ran at 2.5us/stage and dominated. Here exp writes FP16 pT
     and PV runs fp16 x fp16 (same 1 col/cycle PE rate as f32r, no
     f32r-rounding provenance rule in the BIR verifier).
  2. A minimal S->exp->PV chain loop measures 3.5us/iter vs ~1.0us of
     engine work: the in-order PE queue stalls on PV(i) waiting for
     exp(i) (cross-engine sem hops ~0.7us), blocking the independent
     S(i+1) behind it. Fix: EMIT PV lagged LAG=2 k-tiles behind S, so
     the PE stream is [... S(i), PV(i-2), S(i+1), PV(i-1) ...] and
     exp(i) has ~2 iterations of slack to land.
  3. gpsimd.partition_broadcast is ~3.1us for [64,512]: fine for
     throughput on the idle GPSIMD engine but too slow inside the
     ctx-PSUM-release path. The ctx tile is evicted to SBUF by one
     DVE copy first; reciprocal/broadcast/multiply then run off the
     critical path.

Dataflow per pair of units (A, B), per 512-wide q-span gg:
  S^T  : stage [128 k, 512 q] PSUM (1 bank, 6 bufs) = kT-tile^T @ qT
         chunk; the pair's two S matmuls are row-tiled
         (tile_position (0,0)/(64,0)) and run concurrently on the PE.
  exp  : ACT exp(scale*s) -> FP16 pT [128, 512]. No max subtraction:
         scores/8 ~ N(0,1), exp range [e-6, e6] is fp16-safe, softmax
         is shift-invariant.
  PV   : ctx~T [65, 512] += V~[k-tile]^T @ pT (fp16 x fp16, fp32 PSUM
         accumulate); V~ has a host-baked ones column -> row 64 is the
         softmax denominator. 2 ctx tiles (1 bank each).
  norm : DVE-evict ctx -> SBUF (releases the PSUM bank), reciprocal of
         the denominator row, GPSIMD partition_broadcast, tensor_mul,
         DMA out.

V is pre-shuffled on the host to [128, n_ktiles, hn+1] fp16 so its DMA
is contiguous per partition.
"""

import numpy as np
from contextlib import ExitStack

import concourse.bass as bass
import concourse.tile as tile
from concourse import bacc, mybir
from concourse.bass_utils import run_bass_kernel_spmd

F32 = mybir.dt.float32
F32R = mybir.dt.float32r
F16 = mybir.dt.float16
EXP = mybir.ActivationFunctionType.Exp

N_CORES = 8


def build_attention_nc(n_units=8, sq=2048, sk=2048, hn=64,
                       num_devices=N_CORES, loop_iters=1,
                       lag=2, warm_mms=14, ablate=()):
    assert sk % 128 == 0
    n_ktiles = sk // 128
    n_gg = sq // 512  # q-span granularity
    inv_norm = 1.0 / float(np.sqrt(np.float32(hn)))
    assert n_units % 2 == 0

    nc = bacc.Bacc("TRN2", target_bir_lowering=False, debug=False,
                   num_devices=num_devices)

    qT = nc.dram_tensor("qT", [n_units, hn, sq], F32,
                        kind="ExternalInput").ap()
    kT = nc.dram_tensor("kT", [n_units, hn, sq], F32,
                        kind="ExternalInput").ap()
    # host pre-shuffled: v[u, p, t, h] = V[u, t*128+p, h], fp16
    v = nc.dram_tensor("v", [n_units, 128, n_ktiles * (hn + 1)], F16,
                       kind="ExternalInput").ap()
    out = nc.dram_tensor("out", [n_units, hn, sq], F32,
                         kind="ExternalOutput").ap()

    with tile.TileContext(nc) as tc, ExitStack() as ctx:
        qk_pool = ctx.enter_context(tc.tile_pool(name="qk", bufs=4))
        v_pool = ctx.enter_context(tc.tile_pool(name="v", bufs=4))
        p_pool = ctx.enter_context(tc.tile_pool(name="p", bufs=8))
        c_pool = ctx.enter_context(tc.tile_pool(name="c", bufs=4))
        o_pool = ctx.enter_context(tc.tile_pool(name="o", bufs=6))
        r_pool = ctx.enter_context(tc.tile_pool(name="r", bufs=6))
        stage_pool = ctx.enter_context(
            tc.tile_pool(name="stage", bufs=6, space="PSUM"))
        ctxp_pool = ctx.enter_context(
            tc.tile_pool(name="ctxp", bufs=2, space="PSUM"))

        loop_cm = tc.For_i(0, loop_iters, 1) if loop_iters > 1 else None
        if loop_cm is not None:
            loop_cm.__enter__()

        def load_pair(ua):
            # both units stacked on partitions so the two S^T matmuls
            # run as concurrent row-tiles on the PE
            qTp = qk_pool.tile([2 * hn, sq], F32R, tag="qT", name=f"qT{ua}")
            kTp = qk_pool.tile([2 * hn, sq], F32R, tag="kT", name=f"kT{ua}")
            vs = []
            for d in range(2):
                nc.sync.dma_start(qTp[d * hn:(d + 1) * hn, :],
                                  qT[ua + d].bitcast(F32R))
                nc.sync.dma_start(kTp[d * hn:(d + 1) * hn, :],
                                  kT[ua + d].bitcast(F32R))
                v_sb = v_pool.tile([128, n_ktiles, hn + 1], F16, tag="v",
                                   name=f"v{ua + d}")
                nc.sync.dma_start(
                    v_sb[:], v[ua + d].rearrange("p (t h) -> p t h",
                                                 t=n_ktiles))
                vs.append(v_sb)
            return qTp, kTp, vs

        def normalize_and_store(u, gg, ctx_ps):
            # evict promptly: one DVE copy frees the PSUM ctx bank
            q0 = gg * 512
            ctx_sb = c_pool.tile([hn + 1, 512], F32, tag="csb",
                                 name=f"csb{u}_{gg}")
            nc.vector.tensor_copy(ctx_sb[:], ctx_ps[:])
            if "no_norm" in ablate:
                nc.sync.dma_start(out[u, :, q0:q0 + 512],
                                  ctx_sb[0:hn, :])
                return
            rbc = r_pool.tile([1, 512], F32, tag="rbc",
                              name=f"rbc{u}_{gg}")
            nc.vector.reciprocal(rbc[:], ctx_sb[hn:hn + 1, :])
            rb64 = r_pool.tile([hn, 512], F32, tag="rb64",
                               name=f"rb64{u}_{gg}")
            nc.gpsimd.partition_broadcast(rb64[:], rbc[:])
            o_sb = o_pool.tile([hn, 512], F32, tag="o",
                               name=f"o{u}_{gg}")
            nc.vector.tensor_tensor(o_sb[:], ctx_sb[0:hn, :], rb64[:],
                                    mybir.AluOpType.mult)
            nc.sync.dma_start(out[u, :, q0:q0 + 512], o_sb[:])

        pair_tiles = load_pair(0)

        # dense warmup burst to open the PE HAM clock gate
        if warm_mms:
            qTp0, kTp0, _ = pair_tiles
            wstage = [stage_pool.tile([128, 512], F32, tag="stage",
                                      name=f"warm{j}") for j in range(2)]
            for j in range(warm_mms):
                nc.tensor.matmul(wstage[j % 2][:],
                                 kTp0[0:hn, 0:128], qTp0[0:hn, 0:512],
                                 start=True, stop=True)

        for ua in range(0, n_units, 2):
            qTp, kTp, vs = pair_tiles
            if ua + 2 < n_units:
                pair_tiles = load_pair(ua + 2)

            for gg in range(n_gg):
                q0 = gg * 512
                ctxs = [ctxp_pool.tile([hn + 1, 512], F32, tag="ctx",
                                       name=f"ctx{ua + d}_{gg}")
                        for d in range(2)]
                pTs = {}
                for i in range(n_ktiles + lag):
                    if i < n_ktiles:
                        stages = []
                        for d in range(2):
                            stage = stage_pool.tile(
                                [128, 512], F32, tag="stage",
                                name=f"st{ua + d}_{gg}_{i}")
                            if "no_s" in ablate:
                                nc.vector.memset(stage[:, 0:8], 0.0)
                            else:
                                nc.tensor.matmul(
                                    stage[:],
                                    kTp[d * hn:(d + 1) * hn,
                                        i * 128:(i + 1) * 128],
                                    qTp[d * hn:(d + 1) * hn,
                                        q0:q0 + 512],
                                    start=True, stop=True,
                                    tile_position=(d * hn, 0))
                            stages.append(stage)
                        for d in range(2):
                            pT = p_pool.tile([128, 512], F16, tag="pT",
                                             name=f"pT{ua + d}_{gg}_{i}")
                            if "no_exp" in ablate:
                                nc.vector.memset(pT[:], 1.0)
                            else:
                                nc.scalar.activation(pT[:], stages[d][:],
                                                     EXP, scale=inv_norm)
                            pTs[(d, i)] = pT
                    if i >= lag:
                        j = i - lag
                        for d in range(2):
                            if "no_pv" in ablate and j not in (
                                    0, n_ktiles - 1):
                                continue
                            nc.tensor.matmul(
                                ctxs[d][:],
                                vs[d][:, j, :],
                                pTs[(d, j)][:],
                                start=(j == 0), stop=(j == n_ktiles - 1))
                for d in range(2):
                    normalize_and_store(ua + d, gg, ctxs[d])

        if loop_cm is not None:
            loop_cm.__exit__(None, None, None)

    nc.compile()
    return nc


_CACHE = {}


def _get_nc():
    if "nc" not in _CACHE:
        _CACHE["nc"] = build_attention_nc()
    return _CACHE["nc"]


def kernel(query, key, value):
    b, sq, nh, hn = query.shape
    assert (b, sq, nh, hn) == (2, 2048, 32, 64)
    nu = b * nh
    per = nu // N_CORES
    n_ktiles = sq // 128

    qT = np.ascontiguousarray(
        query.transpose(0, 2, 3, 1).reshape(nu, hn, sq)).astype(np.float32)
    kT = np.ascontiguousarray(
        key.transpose(0, 2, 3, 1).reshape(nu, hn, sq)).astype(np.float32)
    # v[u, p, t, h] = V[u, t*128+p, h], ones column at h=hn, fp16
    vv = np.ones((nu, 128, n_ktiles, hn + 1), np.float16)
    vsrc = value.transpose(0, 2, 1, 3).reshape(nu, sq, hn)
    vv[:, :, :, 0:hn] = np.ascontiguousarray(
        vsrc.reshape(nu, n_ktiles, 128, hn).transpose(0, 2, 1, 3)
    ).astype(np.float16)
    vv = vv.reshape(nu, 128, n_ktiles * (hn + 1))

    nc = _get_nc()
    in_maps = [
        {"qT": qT[c * per:(c + 1) * per],
         "kT": kT[c * per:(c + 1) * per],
         "v": vv[c * per:(c + 1) * per]}
        for c in range(N_CORES)
    ]
    res = run_bass_kernel_spmd(nc, in_maps, list(range(N_CORES)))
    ctxo = np.concatenate([res.results[c]["out"] for c in range(N_CORES)],
                          axis=0)  # [nu, hn, sq]
    outp = ctxo.reshape(b, nh, hn, sq).transpose(0, 3, 1, 2)
    return np.ascontiguousarray(outp.reshape(b, sq, nh * hn)).astype(np.float32)
